# revision 5
# baseline (speedup 1.0000x reference)
"""Grouped SwiGLU expert MLP (MoE) on 8 Trainium2 NeuronCores.

Problem: sorted_x [32768, 512] f32, tokens pre-sorted by expert into 8 equal
contiguous segments of 4096 tokens; per-expert SwiGLU MLP
    h12 = x_e @ w12[e].T          (4096, 2816)
    h   = silu(h12[:, :1408]) * h12[:, 1408:]
    out = h @ w3[e].T             (4096, 512)

Sharding: pure expert parallelism — core e owns expert e's weights and its
4096-token segment (sliced host-side from expert_starts), so no device-side
collectives are needed; the host concatenates the per-core outputs.

Device layout is feature-major throughout ("contraction dim on partitions"),
which makes both GEMMs transpose-free on chip:
    xt   = x_e.T   [512, 4096]  bf16
    w12t = w12.T   [512, 2816]  bf16
    w3t  = w3.T    [1408, 512]  bf16
    outT = out.T   [512, 4096]  f32   (host transposes back)
GEMM1 produces H12^T tiles [128h, Nt] (PSUM), SwiGLU runs on ACT+DVE into
bf16 H^T tiles, GEMM2 consumes them directly. bf16 operands run the PE at
1 cycle/row (vs 4 for f32); accumulation is always f32 in PSUM.
"""

import os

import numpy as np
import ml_dtypes

import concourse.bass as bass
import concourse.mybir as mybir
import concourse.tile as tile
from concourse import bacc
from concourse.bass_utils import run_bass_kernel_spmd

N_CORES = 8
D = 512  # d_model
H = 1408  # hidden
TWOH = 2 * H
TPE = 4096  # tokens per expert
NT = 512  # token block (matmul moving free dim, one PSUM bank in f32)
KD = D // 128  # 4 contraction tiles over d
KH = H // 128  # 11 contraction tiles over h
NB = TPE // NT  # token blocks

BF16 = mybir.dt.bfloat16
F32 = mybir.dt.float32
NP_BF16 = np.dtype(ml_dtypes.bfloat16)

# Results of a traced run (test harness reads these).
last_exec_time_ns = None
last_trace_path = None


def _build():
    # Bacc (not plain Bass): its compile() pass pipeline legalizes sync
    # waits (>=2 waits per instruction are split into event-sem chains),
    # which this image's walrus requires.
    nc = bacc.Bacc("TRN2", target_bir_lowering=False, debug=False, num_devices=N_CORES)
    xt = nc.dram_tensor("xt", [D, TPE], BF16, kind="ExternalInput")
    w12t = nc.dram_tensor("w12t", [D, TWOH], BF16, kind="ExternalInput")
    w3t = nc.dram_tensor("w3t", [H, D], BF16, kind="ExternalInput")
    outT = nc.dram_tensor("outT", [D, TPE], F32, kind="ExternalOutput")

    with tile.TileContext(nc) as tc:
        with (
            tc.tile_pool(name="weights", bufs=1) as wpool,
            tc.tile_pool(name="xin", bufs=1) as xpool,
            tc.tile_pool(name="ht", bufs=2) as hpool,
            tc.tile_pool(name="swi", bufs=4) as spool,
            tc.tile_pool(name="ot", bufs=4) as opool,
            tc.tile_pool(name="pg", bufs=2, space=bass.MemorySpace.PSUM) as pgate,
            tc.tile_pool(name="pu", bufs=2, space=bass.MemorySpace.PSUM) as pup,
            tc.tile_pool(name="po", bufs=2, space=bass.MemorySpace.PSUM) as pout,
        ):
            w12s = wpool.tile([128, KD, TWOH], BF16)
            for kd in range(KD):
                nc.sync.dma_start(
                    out=w12s[:, kd, :], in_=w12t[kd * 128 : (kd + 1) * 128, :]
                )
            w3s = wpool.tile([128, KH, D], BF16)
            for kh in range(KH):
                nc.sync.dma_start(
                    out=w3s[:, kh, :], in_=w3t[kh * 128 : (kh + 1) * 128, :]
                )
            xs = xpool.tile([128, KD, TPE], BF16)
            for kd in range(KD):
                for tb in range(NB):
                    nc.sync.dma_start(
                        out=xs[:, kd, tb * NT : (tb + 1) * NT],
                        in_=xt[kd * 128 : (kd + 1) * 128, tb * NT : (tb + 1) * NT],
                    )

            for tb in range(NB):
                tsl = bass.ts(tb, NT)
                ht = hpool.tile([128, KH, NT], BF16)
                for hh in range(KH):
                    ps_g = pgate.tile([128, NT], F32)
                    ps_u = pup.tile([128, NT], F32)
                    for kd in range(KD):
                        nc.tensor.matmul(
                            ps_g[:],
                            w12s[:, kd, hh * 128 : (hh + 1) * 128],
                            xs[:, kd, tsl],
                            start=(kd == 0),
                            stop=(kd == KD - 1),
                        )
                    for kd in range(KD):
                        nc.tensor.matmul(
                            ps_u[:],
                            w12s[:, kd, H + hh * 128 : H + (hh + 1) * 128],
                            xs[:, kd, tsl],
                            start=(kd == 0),
                            stop=(kd == KD - 1),
                        )
                    sil = spool.tile([128, NT], F32)
                    nc.scalar.activation(
                        sil[:], ps_g[:], mybir.ActivationFunctionType.Silu
                    )
                    nc.vector.tensor_mul(ht[:, hh, :], sil[:], ps_u[:])

                for do in range(KD):
                    ps_o = pout.tile([128, NT], F32)
                    for kh in range(KH):
                        nc.tensor.matmul(
                            ps_o[:],
                            w3s[:, kh, do * 128 : (do + 1) * 128],
                            ht[:, kh, :],
                            start=(kh == 0),
                            stop=(kh == KH - 1),
                        )
                    ot = opool.tile([128, NT], F32)
                    nc.scalar.copy(ot[:], ps_o[:])
                    nc.sync.dma_start(
                        out=outT[do * 128 : (do + 1) * 128, tsl], in_=ot[:]
                    )
    nc.compile()
    return nc


_nc_cache = None


def _get_nc():
    global _nc_cache
    if _nc_cache is None:
        _nc_cache = _build()
    return _nc_cache


def kernel(sorted_x, w12, w3, expert_starts, expert_ends):
    global last_exec_time_ns, last_trace_path
    sorted_x = np.asarray(sorted_x)
    w12 = np.asarray(w12)
    w3 = np.asarray(w3)
    starts = np.asarray(expert_starts).astype(np.int64)
    T = sorted_x.shape[0]

    in_maps = []
    for e in range(N_CORES):
        # jax.lax.dynamic_slice clamps the start index the same way
        s = int(min(max(starts[e], 0), T - TPE))
        xe = sorted_x[s : s + TPE]  # (TPE, D) f32
        in_maps.append(
            {
                "xt": np.ascontiguousarray(xe.T).astype(NP_BF16),
                "w12t": np.ascontiguousarray(w12[e].T).astype(NP_BF16),
                "w3t": np.ascontiguousarray(w3[e].T).astype(NP_BF16),
            }
        )

    trace = bool(os.environ.get("BASS_MOE_TRACE"))
    res = run_bass_kernel_spmd(
        _get_nc(), in_maps, core_ids=list(range(N_CORES)), trace=trace
    )
    if trace:
        last_exec_time_ns = res.exec_time_ns
        iat = res.instructions_and_trace
        last_trace_path = iat[1] if iat else None

    out = np.empty((N_CORES * TPE, D), dtype=np.float32)
    for e in range(N_CORES):
        out[e * TPE : (e + 1) * TPE] = res.results[e]["outT"].T
    return out


# revision 7
# speedup vs baseline: 1.0628x; 1.0628x over previous
"""Grouped SwiGLU expert MLP (MoE) on 8 Trainium2 NeuronCores.

Problem: sorted_x [32768, 512] f32, tokens pre-sorted by expert into 8 equal
contiguous segments of 4096 tokens; per-expert SwiGLU MLP
    h12 = x_e @ w12[e].T          (4096, 2816)
    h   = silu(h12[:, :1408]) * h12[:, 1408:]
    out = h @ w3[e].T             (4096, 512)

Sharding: pure expert parallelism — core e owns expert e's weights and its
4096-token segment (sliced host-side from expert_starts), so no device-side
collectives are needed; the host concatenates the per-core outputs.

Device layout is feature-major throughout ("contraction dim on partitions"),
which makes both GEMMs transpose-free on chip:
    xt   = x_e.T   [512, 4096]  bf16
    w12t = w12.T   [512, 2816]  bf16
    w3t  = w3.T    [1408, 512]  bf16
    outT = out.T   [512, 4096]  f32   (host transposes back)
GEMM1 produces H12^T tiles [128h, Nt] (PSUM), SwiGLU runs on ACT+DVE into
bf16 H^T tiles, GEMM2 consumes them directly. bf16 operands run the PE at
1 cycle/row (vs 4 for f32); accumulation is always f32 in PSUM.
"""

import os

import numpy as np
import ml_dtypes

import concourse.bass as bass
import concourse.mybir as mybir
import concourse.tile as tile
from concourse import bacc
from concourse.bass_utils import run_bass_kernel_spmd

N_CORES = 8
D = 512  # d_model
H = 1408  # hidden
TWOH = 2 * H
TPE = 4096  # tokens per expert
NT = 512  # token block (matmul moving free dim, one PSUM bank in f32)
KD = D // 128  # 4 contraction tiles over d
KH = H // 128  # 11 contraction tiles over h
NB = TPE // NT  # token blocks

BF16 = mybir.dt.bfloat16
F32 = mybir.dt.float32
NP_BF16 = np.dtype(ml_dtypes.bfloat16)

# Results of a traced run (test harness reads these).
last_exec_time_ns = None
last_trace_path = None


def _build():
    # Bacc (not plain Bass): its compile() pass pipeline legalizes sync
    # waits (>=2 waits per instruction are split into event-sem chains),
    # which this image's walrus requires.
    nc = bacc.Bacc("TRN2", target_bir_lowering=False, debug=False, num_devices=N_CORES)
    xt = nc.dram_tensor("xt", [D, TPE], BF16, kind="ExternalInput")
    w12t = nc.dram_tensor("w12t", [D, TWOH], BF16, kind="ExternalInput")
    w3t = nc.dram_tensor("w3t", [H, D], BF16, kind="ExternalInput")
    outT = nc.dram_tensor("outT", [D, TPE], F32, kind="ExternalOutput")

    # GEMM2 is software-pipelined into the GEMM1/SwiGLU loop with this lag:
    # in iteration hh we issue the GEMM2 matmuls consuming ht[hh - LAG], so
    # the PE never waits on the ACT+DVE SwiGLU chain (~1.3us behind).
    LAG = 2

    with tile.TileContext(nc) as tc:
        with (
            tc.tile_pool(name="weights", bufs=1) as wpool,
            tc.tile_pool(name="xin", bufs=1) as xpool,
            tc.tile_pool(name="ht", bufs=2) as hpool,
            tc.tile_pool(name="swi", bufs=4) as spool,
            tc.tile_pool(name="ot", bufs=4) as opool,
            tc.tile_pool(name="pg", bufs=2, space=bass.MemorySpace.PSUM) as pgate,
            tc.tile_pool(name="pu", bufs=2, space=bass.MemorySpace.PSUM) as pup,
            tc.tile_pool(name="po", bufs=1, space=bass.MemorySpace.PSUM) as pacc,
        ):
            w12s = wpool.tile([128, KD, TWOH], BF16)
            w3s = wpool.tile([128, KH, D], BF16)
            xs = xpool.tile([128, KD, TPE], BF16)

            def dma_x(kd, tb):
                nc.sync.dma_start(
                    out=xs[:, kd, tb * NT : (tb + 1) * NT],
                    in_=xt[kd * 128 : (kd + 1) * 128, tb * NT : (tb + 1) * NT],
                )

            def dma_w12(kd, c0, c1):
                nc.sync.dma_start(
                    out=w12s[:, kd, c0:c1], in_=w12t[kd * 128 : (kd + 1) * 128, c0:c1]
                )

            # DMA issue order tracks first-block consumption: x(tb=0), then
            # gate/up column chunks pairwise in hh order, w3 (needed from
            # hh=LAG), then the remaining token blocks. Deps are shadow-
            # memory precise, so matmuls start as soon as their slice lands.
            for kd in range(KD):
                dma_x(kd, 0)
            bounds = [0, 384, 768, 1152, 1408]
            for ci in range(4):
                ga, gb = bounds[ci], bounds[ci + 1]
                for kd in range(KD):
                    dma_w12(kd, ga, gb)
                for kd in range(KD):
                    dma_w12(kd, H + ga, H + gb)
                if ci == 0:
                    for kh in range(KH):
                        nc.sync.dma_start(
                            out=w3s[:, kh, :], in_=w3t[kh * 128 : (kh + 1) * 128, :]
                        )
            for tb in range(1, NB):
                for kd in range(KD):
                    dma_x(kd, tb)

            for tb in range(NB):
                tsl = bass.ts(tb, NT)
                ht = hpool.tile([128, KH, NT], BF16)
                acc = [
                    pacc.tile([128, NT], F32, name=f"acc{do}", tag=f"acc{do}")
                    for do in range(KD)
                ]

                def gemm2_step(kh):
                    for do in range(KD):
                        nc.tensor.matmul(
                            acc[do][:],
                            w3s[:, kh, do * 128 : (do + 1) * 128],
                            ht[:, kh, :],
                            start=(kh == 0),
                            stop=(kh == KH - 1),
                        )

                for hh in range(KH):
                    ps_g = pgate.tile([128, NT], F32)
                    ps_u = pup.tile([128, NT], F32)
                    for kd in range(KD):
                        nc.tensor.matmul(
                            ps_g[:],
                            w12s[:, kd, hh * 128 : (hh + 1) * 128],
                            xs[:, kd, tsl],
                            start=(kd == 0),
                            stop=(kd == KD - 1),
                        )
                    for kd in range(KD):
                        nc.tensor.matmul(
                            ps_u[:],
                            w12s[:, kd, H + hh * 128 : H + (hh + 1) * 128],
                            xs[:, kd, tsl],
                            start=(kd == 0),
                            stop=(kd == KD - 1),
                        )
                    sil = spool.tile([128, NT], F32)
                    nc.scalar.activation(
                        sil[:], ps_g[:], mybir.ActivationFunctionType.Silu
                    )
                    nc.vector.tensor_mul(ht[:, hh, :], sil[:], ps_u[:])
                    if hh >= LAG:
                        gemm2_step(hh - LAG)
                for kh in range(KH - LAG, KH):
                    gemm2_step(kh)

                for do in range(KD):
                    ot = opool.tile([128, NT], F32)
                    nc.scalar.copy(ot[:], acc[do][:])
                    nc.sync.dma_start(
                        out=outT[do * 128 : (do + 1) * 128, tsl], in_=ot[:]
                    )
    nc.compile()
    return nc


_nc_cache = None


def _get_nc():
    global _nc_cache
    if _nc_cache is None:
        _nc_cache = _build()
    return _nc_cache


def kernel(sorted_x, w12, w3, expert_starts, expert_ends):
    global last_exec_time_ns, last_trace_path
    sorted_x = np.asarray(sorted_x)
    w12 = np.asarray(w12)
    w3 = np.asarray(w3)
    starts = np.asarray(expert_starts).astype(np.int64)
    T = sorted_x.shape[0]

    in_maps = []
    for e in range(N_CORES):
        # jax.lax.dynamic_slice clamps the start index the same way
        s = int(min(max(starts[e], 0), T - TPE))
        xe = sorted_x[s : s + TPE]  # (TPE, D) f32
        in_maps.append(
            {
                "xt": np.ascontiguousarray(xe.T).astype(NP_BF16),
                "w12t": np.ascontiguousarray(w12[e].T).astype(NP_BF16),
                "w3t": np.ascontiguousarray(w3[e].T).astype(NP_BF16),
            }
        )

    trace = bool(os.environ.get("BASS_MOE_TRACE"))
    res = run_bass_kernel_spmd(
        _get_nc(), in_maps, core_ids=list(range(N_CORES)), trace=trace
    )
    if trace:
        last_exec_time_ns = res.exec_time_ns
        iat = res.instructions_and_trace
        last_trace_path = iat[1] if iat else None

    out = np.empty((N_CORES * TPE, D), dtype=np.float32)
    for e in range(N_CORES):
        out[e * TPE : (e + 1) * TPE] = res.results[e]["outT"].T
    return out


# revision 10
# speedup vs baseline: 1.0744x; 1.0109x over previous
"""Grouped SwiGLU expert MLP (MoE) on 8 Trainium2 NeuronCores.

Problem: sorted_x [32768, 512] f32, tokens pre-sorted by expert into 8 equal
contiguous segments of 4096 tokens; per-expert SwiGLU MLP
    h12 = x_e @ w12[e].T          (4096, 2816)
    h   = silu(h12[:, :1408]) * h12[:, 1408:]
    out = h @ w3[e].T             (4096, 512)

Sharding: pure expert parallelism — core e owns expert e's weights and its
4096-token segment (sliced host-side from expert_starts), so no device-side
collectives are needed; the host concatenates the per-core outputs.

Device layout is feature-major throughout ("contraction dim on partitions"),
which makes both GEMMs transpose-free on chip:
    xt   = x_e.T   [512, 4096]  fp16
    w12t = w12.T   [512, 2816]  fp16
    w3t  = w3.T    [1408, 512]  fp16
    outT = out.T   [512, 4096]  f32   (host transposes back)
GEMM1 produces H12^T tiles [128h, Nt] (PSUM), SwiGLU runs on ACT+DVE into
fp16 H^T tiles, GEMM2 consumes them directly. fp16 operands run the PE at
1 cycle/row (vs 4 for f32) — same speed and footprint as bf16 with a 10-bit
mantissa (8x lower rounding error; inputs here are well inside fp16 range).
Accumulation is always f32 in PSUM.
"""

import os

import numpy as np
import ml_dtypes

import concourse.bass as bass
import concourse.mybir as mybir
import concourse.tile as tile
from concourse import bacc
from concourse.bass_utils import run_bass_kernel_spmd

N_CORES = 8
D = 512  # d_model
H = 1408  # hidden
TWOH = 2 * H
TPE = 4096  # tokens per expert
NT = 512  # token block (matmul moving free dim, one PSUM bank in f32)
KD = D // 128  # 4 contraction tiles over d
KH = H // 128  # 11 contraction tiles over h
NB = TPE // NT  # token blocks

F16 = mybir.dt.float16
F32 = mybir.dt.float32
NP_F16 = np.dtype(np.float16)

# Results of a traced run (test harness reads these).
last_exec_time_ns = None
last_trace_path = None


def _build():
    # Bacc (not plain Bass): its compile() pass pipeline legalizes sync
    # waits (>=2 waits per instruction are split into event-sem chains),
    # which this image's walrus requires.
    nc = bacc.Bacc("TRN2", target_bir_lowering=False, debug=False, num_devices=N_CORES)
    xt = nc.dram_tensor("xt", [D, TPE], F16, kind="ExternalInput")
    w12t = nc.dram_tensor("w12t", [D, TWOH], F16, kind="ExternalInput")
    w3t = nc.dram_tensor("w3t", [H, D], F16, kind="ExternalInput")
    outT = nc.dram_tensor("outT", [D, TPE], F32, kind="ExternalOutput")

    # GEMM2 is software-pipelined into the GEMM1/SwiGLU loop with this lag:
    # in iteration hh we issue the GEMM2 matmuls consuming ht[hh - LAG], so
    # the PE never waits on the ACT+DVE SwiGLU chain (~1.3us behind).
    LAG = 2

    with tile.TileContext(nc) as tc:
        with (
            tc.tile_pool(name="weights", bufs=1) as wpool,
            tc.tile_pool(name="xin", bufs=1) as xpool,
            tc.tile_pool(name="ht", bufs=2) as hpool,
            tc.tile_pool(name="swi", bufs=4) as spool,
            tc.tile_pool(name="ot", bufs=4) as opool,
            tc.tile_pool(name="pg", bufs=2, space=bass.MemorySpace.PSUM) as pgate,
            tc.tile_pool(name="pu", bufs=2, space=bass.MemorySpace.PSUM) as pup,
            tc.tile_pool(name="po", bufs=1, space=bass.MemorySpace.PSUM) as pacc,
        ):
            w12s = wpool.tile([128, KD, TWOH], F16)
            w3s = wpool.tile([128, KH, D], F16)
            xs = xpool.tile([128, KD, TPE], F16)

            def dma_x(kd, tb):
                nc.sync.dma_start(
                    out=xs[:, kd, tb * NT : (tb + 1) * NT],
                    in_=xt[kd * 128 : (kd + 1) * 128, tb * NT : (tb + 1) * NT],
                )

            def dma_w12(kd, c0, c1):
                nc.sync.dma_start(
                    out=w12s[:, kd, c0:c1], in_=w12t[kd * 128 : (kd + 1) * 128, c0:c1]
                )

            # DMA issue order tracks first-block consumption: x(tb=0), then
            # gate/up column chunks pairwise in hh order, w3 (needed from
            # hh=LAG), then the remaining token blocks. Deps are shadow-
            # memory precise, so matmuls start as soon as their slice lands.
            for kd in range(KD):
                dma_x(kd, 0)
            bounds = [0, 384, 768, 1152, 1408]
            w3_order = [(0, 3), (3, 6), (6, 9), (9, 11)]
            for ci in range(4):
                ga, gb = bounds[ci], bounds[ci + 1]
                for kd in range(KD):
                    dma_w12(kd, ga, gb)
                for kd in range(KD):
                    dma_w12(kd, H + ga, H + gb)
                for kh in range(*w3_order[ci]):
                    nc.sync.dma_start(
                        out=w3s[:, kh, :], in_=w3t[kh * 128 : (kh + 1) * 128, :]
                    )
            for tb in range(1, NB):
                for kd in range(KD):
                    dma_x(kd, tb)

            for tb in range(NB):
                tsl = bass.ts(tb, NT)
                ht = hpool.tile([128, KH, NT], F16)
                acc = [
                    pacc.tile([128, NT], F32, name=f"acc{do}", tag=f"acc{do}")
                    for do in range(KD)
                ]

                def gemm2_step(kh):
                    for do in range(KD):
                        nc.tensor.matmul(
                            acc[do][:],
                            w3s[:, kh, do * 128 : (do + 1) * 128],
                            ht[:, kh, :],
                            start=(kh == 0),
                            stop=(kh == KH - 1),
                        )

                for hh in range(KH):
                    ps_g = pgate.tile([128, NT], F32)
                    ps_u = pup.tile([128, NT], F32)
                    for kd in range(KD):
                        nc.tensor.matmul(
                            ps_g[:],
                            w12s[:, kd, hh * 128 : (hh + 1) * 128],
                            xs[:, kd, tsl],
                            start=(kd == 0),
                            stop=(kd == KD - 1),
                        )
                    for kd in range(KD):
                        nc.tensor.matmul(
                            ps_u[:],
                            w12s[:, kd, H + hh * 128 : H + (hh + 1) * 128],
                            xs[:, kd, tsl],
                            start=(kd == 0),
                            stop=(kd == KD - 1),
                        )
                    sil = spool.tile([128, NT], F32)
                    nc.scalar.activation(
                        sil[:], ps_g[:], mybir.ActivationFunctionType.Silu
                    )
                    nc.vector.tensor_mul(ht[:, hh, :], sil[:], ps_u[:])
                    if hh >= LAG:
                        gemm2_step(hh - LAG)
                for kh in range(KH - LAG, KH):
                    gemm2_step(kh)

                for do in range(KD):
                    ot = opool.tile([128, NT], F32)
                    # split PSUM->SBUF copies across ACT and DVE so the final
                    # block's epilogue drains in parallel
                    if do % 2 == 0:
                        nc.scalar.copy(ot[:], acc[do][:])
                    else:
                        nc.vector.tensor_copy(ot[:], acc[do][:])
                    nc.sync.dma_start(
                        out=outT[do * 128 : (do + 1) * 128, tsl], in_=ot[:]
                    )
    nc.compile()
    return nc


_nc_cache = None


def _get_nc():
    global _nc_cache
    if _nc_cache is None:
        _nc_cache = _build()
    return _nc_cache


def kernel(sorted_x, w12, w3, expert_starts, expert_ends):
    global last_exec_time_ns, last_trace_path
    sorted_x = np.asarray(sorted_x)
    w12 = np.asarray(w12)
    w3 = np.asarray(w3)
    starts = np.asarray(expert_starts).astype(np.int64)
    T = sorted_x.shape[0]

    in_maps = []
    for e in range(N_CORES):
        # jax.lax.dynamic_slice clamps the start index the same way
        s = int(min(max(starts[e], 0), T - TPE))
        xe = sorted_x[s : s + TPE]  # (TPE, D) f32
        in_maps.append(
            {
                "xt": np.ascontiguousarray(xe.T).astype(NP_F16),
                "w12t": np.ascontiguousarray(w12[e].T).astype(NP_F16),
                "w3t": np.ascontiguousarray(w3[e].T).astype(NP_F16),
            }
        )

    trace = bool(os.environ.get("BASS_MOE_TRACE"))
    res = run_bass_kernel_spmd(
        _get_nc(), in_maps, core_ids=list(range(N_CORES)), trace=trace
    )
    if trace:
        last_exec_time_ns = res.exec_time_ns
        iat = res.instructions_and_trace
        last_trace_path = iat[1] if iat else None

    out = np.empty((N_CORES * TPE, D), dtype=np.float32)
    for e in range(N_CORES):
        out[e * TPE : (e + 1) * TPE] = res.results[e]["outT"].T
    return out
